# revision 11
# baseline (speedup 1.0000x reference)
"""Trainium2 Bass kernel for nn_MemBlock (dense transformer block).

Reference computation (B=4, T=1024, H=1024, K=16 heads, hd=64):
    h  = LN(x);  q,k,v = h@Wq, h@Wk, h@Wv  (per-head split)
    s  = q k^T / sqrt(hd);  masked (future) positions FILLED with 1e-9 (not -inf)
    a  = softmax(s);  y = a v;  x = x + y
    h2 = LN(x);  out = x + gelu(h2@W1)@W2

Key numerical fact exploited: in fp32, exp(1e-9) == 1.0 exactly, so every
"masked" (strictly-future) position carries softmax weight exp(0)=1.  A fully
masked 128x128 score block contributes plain column-sums of V to the
numerator and a count to the denominator -- folded into the attention-value
accumulation as one extra tiny matmul (suffix^T x block-indicator) per
(head, column-half).  Only lower-triangular blocks of the score matrix are
computed; the diagonal block is masked multiplicatively (s *= tri01) so
masked entries become exp(0)=1, exactly matching the reference.

Sharding (8 cores, SPMD): core c handles batch b=c//2 and head-half h=c%2:
attention over heads [8h, 8h+8) for ALL T rows, then a pairwise exchange
hands each core its own T-row half of the full-width attention output, and
each core runs LN2 + the full-weight MLP on its 512 own rows.

v3 schedule:
  - h^T is produced by XBAR DMA-transposes (one 3D dma_start_transpose per
    row tile on the scalar HWDGE queue) and cast to fp8 -- the PE and the
    vector engine are both out of the LN1 critical path.
  - q/k/v projections run in fp8e4 DoubleRow (weights prescaled x16 on the
    host, 1/16 folded into the PSUM->SBUF copies); scores/AV stay bf16.
  - Attention is exp-bound on the scalar engine, so scalar does ONLY exps
    (+2 staging masks per pair); all copies/masks/finalize run on vector.
  - The y exchange is 4 pieces (one per head pair), ReduceScatter'd behind
    later pairs; z = x+y accumulates per piece, and LN2's bn_stats run per
    piece too, so after the last RS only ~5us of LN2 tail remains.
  - w1/w2 weight DMAs trickle in on the scalar queue during attention;
    late w1 chunks prefetch just-in-time on the sync queue during MLP1.
"""

import numpy as np
import ml_dtypes

import concourse.bass as bass
import concourse.tile as tile
from concourse import bacc, mybir
from concourse.bass_utils import run_bass_kernel_spmd
from concourse.masks import make_identity, make_upper_triangular

F32 = mybir.dt.float32
BF16 = mybir.dt.bfloat16
FP8 = mybir.dt.float8e4
AF = mybir.ActivationFunctionType
ALU = mybir.AluOpType
DR = mybir.MatmulPerfMode.DoubleRow

B, T, H, NK, HD = 4, 1024, 1024, 16, 64
NHC = 8          # heads per core
TO = 512         # own rows per core
FF = 4 * H       # 4096
P = 128
EPS = 1e-5
WS = 16.0        # fp8 weight prescale

REPLICA_GROUPS = [[0, 1], [2, 3], [4, 5], [6, 7]]

_CACHE = {}


def _build_program():
    nc = bacc.Bacc("TRN2", target_bir_lowering=False, debug=False, num_devices=8)

    x_full = nc.dram_tensor("x_full", [T, H], F32, kind="ExternalInput").ap()
    x_own = nc.dram_tensor("x_own", [TO, H], F32, kind="ExternalInput").ap()
    wq = nc.dram_tensor("wq", [H, NHC * HD], FP8, kind="ExternalInput").ap()
    wk = nc.dram_tensor("wk", [H, NHC * HD], FP8, kind="ExternalInput").ap()
    wv = nc.dram_tensor("wv", [H, NHC * HD], FP8, kind="ExternalInput").ap()
    w1 = nc.dram_tensor("w1", [H, FF], BF16, kind="ExternalInput").ap()
    w2 = nc.dram_tensor("w2", [FF, H], BF16, kind="ExternalInput").ap()
    sel = nc.dram_tensor("sel", [1, 2], F32, kind="ExternalInput").ap()
    bind = nc.dram_tensor("bind", [8, T], BF16, kind="ExternalInput").ap()
    out = nc.dram_tensor("out", [TO, H], F32, kind="ExternalOutput").ap()

    cc_in = [nc.dram_tensor(f"cc_in{p}", [2, TO, 256], BF16) for p in range(4)]
    cc_out = [nc.dram_tensor(f"cc_out{p}", [TO, 256], BF16) for p in range(4)]

    with tile.TileContext(nc) as tc:
        with tc.tile_pool(name="consts", bufs=1) as consts, \
             tc.tile_pool(name="persist", bufs=1) as persist, \
             tc.tile_pool(name="w1pool", bufs=2) as w1pool:

            ident = consts.tile([P, P], F32)
            make_identity(nc, ident)
            tri = consts.tile([P, P], F32)  # tri[p,t] = 1 if p <= t else 0
            make_upper_triangular(nc, tri, val=1.0, diag=True)
            eps_t = consts.tile([P, 1], F32)
            nc.vector.memset(eps_t, EPS)
            # ind[p, i, j] = 1 if i > j else 0 (suffix-of-blocks indicator)
            ind = consts.tile([P, 8, 8], BF16)
            nc.vector.memset(ind, 0.0)
            for i in range(1, 8):
                nc.vector.memset(ind[:, i, 0:i], 1.0)
            # blockind[j, q] = 1 if q//128 == j (suffix broadcast MM rhs)
            blockind = consts.tile([8, T], BF16)
            nc.gpsimd.dma_start(out=blockind, in_=bind)
            sel_sb = consts.tile([P, 2], F32)
            nc.gpsimd.dma_start(
                out=sel_sb,
                in_=bass.AP(tensor=sel.tensor, offset=0, ap=[[0, P], [1, 2]]),
            )

            x_own_sb = persist.tile([P, 4, H], F32)  # becomes z = x+y, then out
            w2_sb = persist.tile([P, 32, H], BF16)
            stats2 = persist.tile([P, 4, 8, 6], F32)  # LN2 per-piece stats

            with tc.tile_pool(name="attn_big", bufs=1) as big, \
                 tc.tile_pool(name="epool", bufs=3) as epool, \
                 tc.tile_pool(name="small", bufs=2) as small, \
                 tc.tile_pool(name="stgpool", bufs=2) as stgpool, \
                 tc.tile_pool(name="zpool", bufs=2) as zpool, \
                 tc.tile_pool(name="ln", bufs=2) as ln, \
                 tc.tile_pool(name="ps_score", bufs=3, space="PSUM") as ps_score, \
                 tc.tile_pool(name="ps_yaug", bufs=2, space="PSUM") as ps_yaug, \
                 tc.tile_pool(name="ps_tr", bufs=2, space="PSUM") as ps_tr, \
                 tc.tile_pool(name="ps_qkv", bufs=1, space="PSUM") as ps_qkv:

                hT8 = big.tile([P, 8, T], FP8)
                qT = big.tile([P, 4, T], BF16)
                kT = big.tile([P, 4, T], BF16)
                v_aug = big.tile([P, 8, NHC, HD + 1], BF16)
                wq_sb = big.tile([P, 8, NHC * HD], FP8)
                wk_sb = big.tile([P, 8, NHC * HD], FP8)
                wv_sb = big.tile([P, 8, NHC * HD], FP8)
                nc.gpsimd.dma_start(out=wv_sb, in_=wv.rearrange("(o p) j -> p o j", p=P))
                nc.gpsimd.dma_start(out=wq_sb, in_=wq.rearrange("(o p) j -> p o j", p=P))
                nc.gpsimd.dma_start(out=wk_sb, in_=wk.rearrange("(o p) j -> p o j", p=P))
                nc.gpsimd.dma_start(x_own_sb, x_own.rearrange("(o p) f -> p o f", p=P))

                # ---- Phase 1: LN1 (vector) + XBAR transpose (DMA) + fp8
                # cast (scalar) + V projection (PE, DoubleRow) per row tile.
                for tt in range(8):
                    xt = ln.tile([P, H], F32, tag="xt")
                    nc.sync.dma_start(xt, x_full[tt * P:(tt + 1) * P, :])
                    stats = ln.tile([P, 2, 6], F32, tag="stats")
                    nc.vector.bn_stats(stats[:, 0, :], xt[:, 0:512])
                    nc.vector.bn_stats(stats[:, 1, :], xt[:, 512:1024])
                    mv = ln.tile([P, 2], F32, tag="mv")
                    nc.vector.bn_aggr(mv, stats)
                    sq = ln.tile([P, 1], F32, tag="sq")
                    nc.scalar.activation(sq, mv[:, 1:2], AF.Sqrt, bias=eps_t[:, 0:1])
                    rstd = ln.tile([P, 1], F32, tag="rstd")
                    nc.vector.reciprocal(rstd, sq)
                    h = ln.tile([P, H], BF16, tag="h")
                    nc.vector.tensor_scalar(
                        h, xt, mv[:, 0:1], rstd, ALU.subtract, ALU.mult
                    )
                    httmp = ln.tile([P, 8, P], BF16, tag="httmp")
                    nc.scalar.dma_start_transpose(httmp, h)
                    nc.scalar.copy(
                        out=hT8[:, :, tt * P:(tt + 1) * P], in_=httmp
                    )
                    # V projection for this row tile (all 8 heads), DoubleRow
                    ps = ps_qkv.tile([P, 512], F32, tag="qkv")
                    for kk in range(4):
                        nc.tensor.matmul(
                            ps,
                            lhsT=hT8[:, 2 * kk:2 * kk + 2, tt * P:(tt + 1) * P],
                            rhs=wv_sb[:, 2 * kk:2 * kk + 2, :],
                            start=(kk == 0),
                            stop=(kk == 3),
                            perf_mode=DR,
                        )
                    nc.scalar.mul(
                        v_aug[:, tt, :, 0:HD],
                        ps.rearrange("p (h d) -> p h d", h=NHC),
                        1.0 / WS,
                    )
                nc.vector.memset(v_aug[:, :, :, HD:HD + 1], 1.0)

                # Early weight DMAs on the scalar queue (first two w1 chunks
                # + w2); the rest go just-in-time on sync during MLP1.
                w1c = [w1pool.tile([P, 8, 512], BF16, tag="w1c", name=f"w1c{i}")
                       for i in range(8)]

                def w1_dma(eng, i):
                    eng.dma_start(
                        w1c[i],
                        w1[:, i * 512:(i + 1) * 512].rearrange(
                            "(o p) f -> p o f", p=P
                        ),
                    )

                w1_dma(nc.scalar, 0)
                w1_dma(nc.scalar, 1)
                nc.scalar.dma_start(w2_sb, w2.rearrange("(o p) n -> p o n", p=P))

                def qk_proj(jt, ch):
                    """q^T,k^T for head pair jt, T-column half ch (fp8 DR)."""
                    for dst, w_sb in ((qT, wq_sb), (kT, wk_sb)):
                        ps = ps_qkv.tile([P, 512], F32, tag="qkv")
                        for kk in range(4):
                            nc.tensor.matmul(
                                ps,
                                lhsT=w_sb[:, 2 * kk:2 * kk + 2,
                                          jt * P:(jt + 1) * P],
                                rhs=hT8[:, 2 * kk:2 * kk + 2,
                                        ch * 512:(ch + 1) * 512],
                                start=(kk == 0),
                                stop=(kk == 3),
                                perf_mode=DR,
                            )
                        nc.vector.tensor_scalar_mul(
                            dst[:, jt, ch * 512:(ch + 1) * 512], ps, 1.0 / WS
                        )

                qk_proj(0, 0)
                qk_proj(0, 1)

                # ---- Phase 2: attention per head pair, pipelined with the
                # next pair's q/k projection and the per-pair exchange ----
                for jt in range(4):
                    pair = (2 * jt, 2 * jt + 1)

                    # sufT[j, d] = sum_{i>j} colsum(V_aug_i)[d]: [8, 65]/head
                    sufp = ps_score.tile([P, 512], F32, tag="sc")
                    for z, h_ in enumerate(pair):
                        for i in range(1, 8):
                            nc.tensor.matmul(
                                sufp[0:8, 65 * z:65 * z + 65],
                                lhsT=ind[:, i, :],
                                rhs=v_aug[:, i, h_, :],
                                start=(i == 1),
                                stop=(i == 7),
                                skip_group_check=True,
                            )
                    sufT_sb = small.tile([8, 130], BF16, tag="sufT")
                    nc.vector.tensor_copy(out=sufT_sb, in_=sufp[0:8, 0:130])

                    stg = stgpool.tile([P, 8, P], BF16, tag="stg")

                    for c in range(2):
                        yaugs = [
                            ps_yaug.tile([HD + 1, 512], F32, tag="yaug",
                                         name=f"yaug{z}")
                            for z in range(2)
                        ]
                        # chain start: suffix broadcast into the yaug banks
                        for z in range(2):
                            nc.tensor.matmul(
                                yaugs[z],
                                lhsT=sufT_sb[0:8, 65 * z:65 * z + 65],
                                rhs=blockind[0:8, 512 * c:512 * (c + 1)],
                                start=True,
                                stop=False,
                                skip_group_check=True,
                            )
                        ilist = [i for i in range(8) if 512 * (c + 1) - 128 * i > 0]
                        for idx, i in enumerate(ilist):
                            sc = max(0, 128 * i - 512 * c)
                            n = 512 - sc
                            sps = {}
                            for z in range(2):
                                sp = ps_score.tile([P, 512], F32, tag="sc",
                                                   name=f"sp{z}")
                                nc.tensor.matmul(
                                    sp[:, :n],
                                    lhsT=kT[64 * z:64 * z + 64, jt,
                                            P * i:P * (i + 1)],
                                    rhs=qT[64 * z:64 * z + 64, jt,
                                           512 * c + sc:512 * (c + 1)],
                                    start=True,
                                    stop=True,
                                )
                                sps[z] = sp
                            for z, h_ in enumerate(pair):
                                sp = sps[z]
                                if 4 * c <= i <= 4 * c + 3:
                                    nc.vector.tensor_tensor(
                                        sp[:, 0:P], sp[:, 0:P], tri, op=ALU.mult
                                    )
                                e = epool.tile([P, 512], BF16, tag="e")
                                nc.scalar.activation(e[:, :n], sp[:, :n], AF.Exp)
                                nc.tensor.matmul(
                                    yaugs[z][:, sc:512],
                                    lhsT=v_aug[:, i, h_, :],
                                    rhs=e[:, :n],
                                    start=False,
                                    stop=(idx == len(ilist) - 1),
                                    skip_group_check=True,
                                )
                        for z, h_ in enumerate(pair):
                            ya_sb = small.tile([HD + 1, 512], F32, tag="ya")
                            nc.vector.tensor_copy(out=ya_sb, in_=yaugs[z])
                            for j2 in range(4):
                                tb = 4 * c + j2
                                yt = ps_tr.tile([P, P], F32, tag="tr")
                                nc.tensor.transpose(
                                    yt[:, :HD + 1],
                                    ya_sb[:, P * j2:P * (j2 + 1)],
                                    ident[:HD + 1, :HD + 1],
                                )
                                rden = small.tile([P, 1], F32, tag="rden")
                                nc.vector.reciprocal(rden, yt[:, HD:HD + 1])
                                nc.vector.tensor_scalar_mul(
                                    stg[:, tb, HD * z:HD * (z + 1)],
                                    yt[:, 0:HD],
                                    rden,
                                )
                        # interleave next pair's q/k projection chunk
                        if jt < 3:
                            qk_proj(jt + 1, c)

                    # stage piece jt: own 128 columns scaled by sel (1 or 0)
                    # into each window half (scalar), then DMA out and RS.
                    stg2 = stgpool.tile([P, 8, 256], BF16, tag="stg2")
                    nc.scalar.mul(stg2[:, :, 0:128], stg, sel_sb[:, 0:1])
                    nc.scalar.mul(stg2[:, :, 128:256], stg, sel_sb[:, 1:2])
                    nc.sync.dma_start(
                        cc_in[jt].rearrange("s (rr p) w -> p (s rr) w", p=P),
                        stg2,
                    )
                    nc.gpsimd.collective_compute(
                        "ReduceScatter",
                        ALU.add,
                        ins=[cc_in[jt][:]],
                        outs=[cc_out[jt][:]],
                        replica_groups=REPLICA_GROUPS,
                    )
                    # receive piece, add into the resident fp32 x, and run
                    # LN2's bn_stats for the two finished column groups.
                    zp = zpool.tile([P, 4, 2, P], BF16, tag="zp")
                    for r in range(2):
                        nc.gpsimd.dma_start(
                            zp[:, :, r, :],
                            cc_out[jt].rearrange(
                                "(o p) (r w) -> p o r w", p=P, r=2)[:, :, r, :],
                        )
                    for r in range(2):
                        g0 = r * 512 + jt * P
                        nc.vector.tensor_tensor(
                            x_own_sb.rearrange("p o (r g w) -> p o r g w",
                                               r=2, g=4)[:, :, r, jt, :],
                            x_own_sb.rearrange("p o (r g w) -> p o r g w",
                                               r=2, g=4)[:, :, r, jt, :],
                            zp[:, :, r, :],
                            op=ALU.add,
                        )
                        for tb in range(4):
                            nc.vector.bn_stats(
                                stats2[:, tb, 2 * jt + r, :],
                                x_own_sb[:, tb, g0:g0 + P],
                            )

            # ---- Phase 3: LN2 + MLP on own rows ----
            with tc.tile_pool(name="mlp_big", bufs=1) as mbig, \
                 tc.tile_pool(name="ln2", bufs=2) as ln2, \
                 tc.tile_pool(name="ps_mm", bufs=3, space="PSUM") as ps_mm, \
                 tc.tile_pool(name="ps_tr2", bufs=2, space="PSUM") as ps_tr2:

                h2T = mbig.tile([P, 8, TO], BF16)
                gT = mbig.tile([P, 32, TO], BF16)

                for tb in range(4):
                    mv = ln2.tile([P, 2], F32, tag="mv2")
                    nc.vector.bn_aggr(mv, stats2[:, tb, :, :])
                    sq = ln2.tile([P, 1], F32, tag="sq2")
                    nc.scalar.activation(sq, mv[:, 1:2], AF.Sqrt, bias=eps_t[:, 0:1])
                    rstd = ln2.tile([P, 1], F32, tag="rstd2")
                    nc.vector.reciprocal(rstd, sq)
                    h2 = ln2.tile([P, H], F32, tag="h2")
                    nc.vector.tensor_scalar(
                        h2, x_own_sb[:, tb, :], mv[:, 0:1], rstd,
                        ALU.subtract, ALU.mult,
                    )
                    for hi in range(8):
                        pt = ps_tr2.tile([P, P], F32, tag="tr2")
                        nc.tensor.transpose(pt, h2[:, hi * P:(hi + 1) * P], ident)
                        nc.scalar.copy(
                            out=h2T[:, hi, tb * P:(tb + 1) * P], in_=pt
                        )

                # MLP1: FF chunk outer (so each w1 chunk is used once and the
                # pool's double-buffering prefetches the next chunk).
                for wc in range(8):
                    for ft in range(4):
                        f = wc * 4 + ft
                        for tbc in range(2):
                            ps = ps_mm.tile([P, 512], F32, tag="mm", name="psg")[:, :256]
                            for hi in range(8):
                                nc.tensor.matmul(
                                    ps,
                                    lhsT=w1c[wc][:, hi, ft * P:(ft + 1) * P],
                                    rhs=h2T[:, hi, 256 * tbc:256 * (tbc + 1)],
                                    start=(hi == 0),
                                    stop=(hi == 7),
                                )
                            nc.scalar.activation(
                                gT[:, f, 256 * tbc:256 * (tbc + 1)],
                                ps, AF.Gelu,
                            )
                    if wc == 1:
                        # just-in-time prefetch of the remaining w1 chunks
                        for i in range(2, 8):
                            w1_dma(nc.sync, i)

                out_r = out.rearrange("(o p) f -> p o f", p=P)
                for tb in range(4):
                    for ch in range(2):
                        ps = ps_mm.tile([P, 512], F32, tag="mm")
                        for ft in range(32):
                            nc.tensor.matmul(
                                ps,
                                lhsT=gT[:, ft, tb * P:(tb + 1) * P],
                                rhs=w2_sb[:, ft, ch * 512:(ch + 1) * 512],
                                start=(ft == 0),
                                stop=(ft == 31),
                            )
                        nc.vector.tensor_tensor(
                            x_own_sb[:, tb, ch * 512:(ch + 1) * 512],
                            x_own_sb[:, tb, ch * 512:(ch + 1) * 512],
                            ps, op=ALU.add,
                        )
                    nc.sync.dma_start(out_r[:, tb, :], x_own_sb[:, tb, :])

    nc.compile()
    return nc


def kernel(**inputs):
    """Full-input / full-output entry point.  See module docstring."""
    if "nc" not in _CACHE:
        _CACHE["nc"] = _build_program()
    nc = _CACHE["nc"]

    E4M3 = ml_dtypes.float8_e4m3

    def q8(a):
        return np.clip(np.asarray(a, np.float32), -240, 240).astype(E4M3)

    x = np.asarray(inputs["x"], np.float32)
    scale = 1.0 / np.sqrt(HD)
    wq_np = q8(np.asarray(inputs["Wq"], np.float32) * (scale * WS))
    wk_np = q8(np.asarray(inputs["Wk"], np.float32) * WS)
    wv_np = q8(np.asarray(inputs["Wv"], np.float32) * WS)
    w1_np = np.asarray(inputs["W1"], np.float32).astype(ml_dtypes.bfloat16)
    w2_np = np.asarray(inputs["W2"], np.float32).astype(ml_dtypes.bfloat16)
    bind_np = np.kron(np.eye(8, dtype=np.float32),
                      np.ones((1, P), np.float32)).astype(ml_dtypes.bfloat16)

    in_maps = []
    for c in range(8):
        b, half = c // 2, c % 2
        cols = slice(half * 512, (half + 1) * 512)
        in_maps.append({
            "x_full": np.ascontiguousarray(x[b]),
            "x_own": np.ascontiguousarray(x[b, half * TO:(half + 1) * TO]),
            "wq": np.ascontiguousarray(wq_np[:, cols]),
            "wk": np.ascontiguousarray(wk_np[:, cols]),
            "wv": np.ascontiguousarray(wv_np[:, cols]),
            "w1": w1_np,
            "w2": w2_np,
            "sel": np.array([[1.0, 0.0]] if half == 0 else [[0.0, 1.0]],
                            np.float32),
            "bind": bind_np,
        })

    res = run_bass_kernel_spmd(nc, in_maps, core_ids=list(range(8)))
    _CACHE["last_results"] = res

    out = np.empty((B, T, H), np.float32)
    for c in range(8):
        b, half = c // 2, c % 2
        out[b, half * TO:(half + 1) * TO] = res.results[c]["out"]
    return out
